# revision 30
# baseline (speedup 1.0000x reference)
"""FAVOR+ attention (Performer) Trainium2 kernel, 8-way sharded.

Sharding: 8 cores = 4 batches x 2 head-groups. Core c handles batch c//2 and
heads [8*(c%2), 8*(c%2)+8). The attention core (kv state) is fully local per
head; the output projection is computed as a per-core partial over its 512
input channels and the two partials per batch are summed on the host.

All three input GEMMs (q-features, k-features, v) run in fp8e4 DoubleRow
(2 rows/cell -> half the matmuls of bf16); x is quantized once on the host
and kept resident in SBUF for both passes. The attention-out GEMM is folded
into the projection on device: wkvp = blockdiag(kv)^T-free form kv @ Wp plus
the rank-1 v-bias term ksum (x) (bv@Wp), so pass B is just
q' -> normalize -> one fp16 GEMM against wkvp. fp16 (not bf16) is used for
all 16-bit intermediates (same PE speed, 8x finer mantissa).
"""

import numpy as np
import ml_dtypes

import concourse.mybir as mybir
import concourse.tile as tile
from concourse import bacc
from concourse.bass_utils import run_bass_kernel_spmd

F32 = mybir.dt.float32
F16 = mybir.dt.float16
FP8 = mybir.dt.float8e4
AF = mybir.ActivationFunctionType
ALU = mybir.AluOpType
DR = mybir.MatmulPerfMode.DoubleRow

N = 4096
D = 1024
HD = 64
NF = 64
EPS = 1e-4
BLK = 512  # n-block
NBLK = N // BLK
NCH = BLK // 128  # 128-row chunks per block
SCALE = float(HD) ** -0.25
WS = 128.0  # fp8 weight pre-scale (undone on the way out of PSUM)


def _build_nc():
    nc = bacc.Bacc("TRN2", target_bir_lowering=False, debug=False, num_devices=8)

    xt8 = nc.dram_tensor("xt8", [NBLK, 128, 8, BLK], FP8, kind="ExternalInput").ap()
    wqp8 = nc.dram_tensor("wqp8", [128, 8, 512], FP8, kind="ExternalInput").ap()
    wkp8 = nc.dram_tensor("wkp8", [128, 8, 512], FP8, kind="ExternalInput").ap()
    wv8 = nc.dram_tensor("wv8", [128, 8, 512], FP8, kind="ExternalInput").ap()
    wp = nc.dram_tensor("wp", [128, 4, 1024], F16, kind="ExternalInput").ap()
    bqpe = nc.dram_tensor("bqpe", [128, 4], F32, kind="ExternalInput").ap()
    bkpb = nc.dram_tensor("bkpb", [128, 8, 64], F16, kind="ExternalInput").ap()
    ident = nc.dram_tensor("ident", [128, 128], F16, kind="ExternalInput").ap()
    out = nc.dram_tensor("out", [D, N], F16, kind="ExternalOutput").ap()

    out_v = out.rearrange("(oc p) n -> p oc n", p=128)  # [128, 8, 4096]

    with tile.TileContext(nc) as tc:
        with (
            tc.tile_pool(name="consts", bufs=1) as consts,
            tc.tile_pool(name="x8p", bufs=NBLK) as x8p,
            tc.tile_pool(name="work", bufs=2) as work,
            tc.tile_pool(name="small", bufs=4) as small,
            tc.tile_pool(name="pbig", bufs=6, space="PSUM") as pbig,
        ):
            pkv = tc.alloc_tile_pool(name="pkv", bufs=1, space="PSUM")
            # ---- pass-A-critical loads, split across 4 DMA rings so the
            # first k-feature matmul's inputs land in ~2 ring-transfers
            # (per-ring bandwidth is ~110 GB/s; rings run in parallel).
            wtmp = consts.tile([128, 128], F16, name="wtmp")
            nc.vector.memset(wtmp[:], 0.125)
            eps_sb = consts.tile([128, 1], F32, name="eps_sb")
            nc.vector.memset(eps_sb[:], EPS)
            wkp8_sb = consts.tile([128, 8, 512], FP8, name="wkp8_sb")
            nc.scalar.dma_start(wkp8_sb[:, 0:4, :], wkp8[:, 0:4, :])
            x80 = x8p.tile([128, 8, BLK], FP8, name="x8_t", tag="x8")
            nc.gpsimd.dma_start(x80[:, 4:8, :], xt8[0][:, 4:8, :])
            nc.gpsimd.dma_start(wkp8_sb[:, 4:8, :], wkp8[:, 4:8, :])
            bkpb_sb = consts.tile([128, 8, 64], F16, name="bkpb_sb")
            nc.scalar.dma_start(bkpb_sb[:], bkpb)
            wv8_sb = consts.tile([128, 8, 512], FP8, name="wv8_sb")
            nc.scalar.dma_start(wv8_sb[:], wv8)
            ident_sb = consts.tile([128, 128], F16, name="ident_sb")
            nc.scalar.dma_start(ident_sb[:], ident)

            # pass-B constants ride the scalar ring AFTER the pass-A
            # critical set — ring FIFO keeps them from stealing bandwidth
            # from the transfers the first matmuls block on.
            wqp8_sb = consts.tile([128, 8, 512], FP8, name="wqp8_sb")
            nc.scalar.dma_start(wqp8_sb[:], wqp8)
            wp_sb = consts.tile([128, 4, 1024], F16, name="wp_sb")
            nc.scalar.dma_start(wp_sb[:], wp)
            bqpe_sb = consts.tile([128, 4], F32, name="bqpe_sb")
            nc.scalar.dma_start(bqpe_sb[:], bqpe)

            # kv accumulators: pairs (0,1) in kvacc0, (2,3) in kvacc1.
            # Layout per pair: 129 cols (64 v-head0 | 64 v-head1 | ksum), stride 130.
            kvacc = [
                pkv.tile([128, 260], F32, name=f"kvacc{t}", tag=f"kvacc{t}")
                for t in range(2)
            ]

            # PE warmup: keep the HAM activity window busy while the first
            # DMAs stream, so real matmuls start at 2.4 GHz.
            ps_warm = pbig.tile([128, 512], F32, name="ps_warm", tag="big")
            for _ in range(34):
                nc.tensor.matmul(
                    ps_warm[:, 0:128], wtmp[:], wtmp[:], start=True, stop=True
                )

            # ================= pass A: k', v -> kv, ksum =================
            def load_x8(blk):
                t = x8p.tile([128, 8, BLK], FP8, name="x8_t", tag="x8")
                nc.sync.dma_start(t[:], xt8[blk])
                return t

            def emit_v(x8_t, c, v_sbs):
                cs = slice(c * 128, (c + 1) * 128)
                psv = pbig.tile([128, 512], F32, name="ps_v", tag="big")
                for k in range(4):
                    nc.tensor.matmul(
                        psv[:],
                        x8_t[:, 2 * k : 2 * k + 2, cs],
                        wv8_sb[:, 2 * k : 2 * k + 2, :],
                        start=(k == 0),
                        stop=(k == 3),
                        perf_mode=DR,
                    )
                v_sb = work.tile([128, 4, 132], F16, name="v_sb", tag="v", bufs=5)
                nc.scalar.activation(
                    v_sb[:, :, 0:128],
                    psv.rearrange("p (g j) -> p g j", j=128),
                    AF.Copy,
                    scale=1.0 / WS,
                )
                nc.vector.memset(v_sb[:, :, 128:129], 1.0)
                v_sbs.append(v_sb)

            def emit_kf(x8_t, c, kp_sbs):
                cs = slice(c * 128, (c + 1) * 128)
                psf = pbig.tile([128, 512], F32, name="ps_kf", tag="big")
                for k in range(4):
                    nc.tensor.matmul(
                        psf[:],
                        x8_t[:, 2 * k : 2 * k + 2, cs],
                        wkp8_sb[:, 2 * k : 2 * k + 2, :],
                        start=(k == 0),
                        stop=(k == 3),
                        perf_mode=DR,
                    )
                psf_v = psf.rearrange("p (g f) -> p g f", f=64)  # [128, 8, 64]
                karg = small.tile([128, 8, 64], F16, name="karg", tag="karg")
                nc.vector.tensor_tensor(karg[:], psf_v, bkpb_sb[:], ALU.add)
                mx = small.tile([128, 8], F32, name="mx", tag="mx")
                nc.vector.reduce_max(mx[:], karg[:], axis=mybir.AxisListType.X)
                nc.gpsimd.tensor_tensor(
                    karg[:], karg[:],
                    mx[:, :, None].to_broadcast([128, 8, 64]),
                    ALU.subtract,
                )
                kp_sb = work.tile([128, 4, 128], F16, name="kp_sb", tag="kp", bufs=9)
                nc.scalar.activation(
                    kp_sb.rearrange("p g (h f) -> p (g h) f", f=64),
                    karg[:], AF.Exp, bias=eps_sb[:], scale=1.0 / WS,
                )
                kp_sbs.append(kp_sb)

            def emit_kv(blk, c, kp_sbs, v_sbs):
                glob_first = blk == 0 and c == 0
                glob_last = blk == NBLK - 1 and c == NCH - 1
                for p in range(4):
                    base = (p % 2) * 130
                    nc.tensor.matmul(
                        kvacc[p // 2][:, base : base + 129],
                        kp_sbs[c][:, p, :],
                        v_sbs[c][:, p, 0:129],
                        start=(glob_first and p % 2 == 0),
                        stop=(glob_last and p % 2 == 1),
                    )

            # blocks 0+1: k-features first (only need wkp8 + x8), so the PE
            # has work while wv8 is still streaming on the scalar ring.
            nc.sync.dma_start(x80[:, 0:4, :], xt8[0][:, 0:4, :])
            x8_blks = [x80, load_x8(1), load_x8(2), load_x8(3)]
            kp01 = [[], []]
            v01 = [[], []]
            for b in range(2):
                for c in range(NCH):
                    emit_kf(x8_blks[b], c, kp01[b])
            for b in range(2):
                for c in range(NCH):
                    emit_v(x8_blks[b], c, v01[b])
                for c in range(NCH):
                    emit_kv(b, c, kp01[b], v01[b])

            for blk in range(2, NBLK):
                if blk + 2 < NBLK:
                    x8_blks.append(load_x8(blk + 2))
                x8_t = x8_blks[blk]
                v_sbs, kp_sbs = [], []
                for c in range(NCH):
                    emit_kf(x8_t, c, kp_sbs)
                    emit_v(x8_t, c, v_sbs)
                for c in range(NCH):
                    emit_kv(blk, c, kp_sbs, v_sbs)

            # ======== boundary: ksum columns + wkvp = kv @ Wp fold ========
            # Pre-queue the first two blocks' q-feature GEMMs so the PE has
            # work while DVE/ACT assemble ksbc/wkvp.
            def emit_qp(blk):
                x8_t = x8_blks[blk]
                qp_sb = work.tile([128, 4, BLK], F16, name="qp_sb", tag="qp", bufs=6)
                for p in range(4):
                    ps = pbig.tile([128, BLK], F32, name="ps_qt", tag="big")
                    for k in range(4):
                        nc.tensor.matmul(
                            ps[:],
                            wqp8_sb[:, 2 * k : 2 * k + 2, p * 128 : (p + 1) * 128],
                            x8_t[:, 2 * k : 2 * k + 2, :],
                            start=(k == 0),
                            stop=(k == 3),
                            perf_mode=DR,
                        )
                    nc.scalar.activation(
                        qp_sb[:, p, :], ps[:], AF.Exp,
                        bias=bqpe_sb[:, p : p + 1], scale=1.0 / WS,
                    )
                return qp_sb

            qps = [emit_qp(0), emit_qp(1), emit_qp(2), emit_qp(3)]

            # ksbc_ext[hf, p, j]: ksum[hf] masked to head(j)'s block, the
            # stationary operand of the merged normalizer+broadcast matmul.
            ksbc = consts.tile([128, 4, 128], F16, name="ksbc")
            nc.vector.memset(ksbc[:], 0.0)
            for p in range(4):
                t = kvacc[p // 2]
                base = (p % 2) * 130
                ks = t[:, base + 128 : base + 129]
                nc.vector.tensor_copy(
                    out=ksbc[0:64, p, 0:64], in_=ks[0:64].to_broadcast([64, 64])
                )
                nc.vector.tensor_copy(
                    out=ksbc[64:128, p, 64:128], in_=ks[64:128].to_broadcast([64, 64])
                )

            # wkvp[hf, p, od] = sum_{vd in head(hf)} kv[hf,vd] Wp[vd,od].
            # (The v-bias needs no device-side term: softmax weights sum to
            # one, so bv contributes the constant bv@Wp, folded into b_proj
            # on the host.)  Stage-ordered so all kvacc reads finish first
            # and the per-pair chains pipeline across engines.
            wkvp_sb = consts.tile([128, 4, 1024], F16, name="wkvp_sb")
            kvsbs, psTs, kvTs = [], [], []
            for p in range(4):
                t = kvacc[p // 2]
                base = (p % 2) * 130
                # block-diagonal extract (off-diag quadrants are cross-head
                # garbage from the full outer-product accumulation)
                kvsb = small.tile([128, 128], F16, name="kvsb", tag="kvsb", bufs=4)
                nc.vector.memset(kvsb[:], 0.0)
                nc.vector.tensor_copy(out=kvsb[0:64, 0:64], in_=t[0:64, base : base + 64])
                nc.vector.tensor_copy(
                    out=kvsb[64:128, 64:128], in_=t[64:128, base + 64 : base + 128]
                )
                kvsbs.append(kvsb)
            for p in range(4):
                psT = pbig.tile([128, 128], F16, name="ps_T", tag="big")
                nc.tensor.transpose(psT[:], kvsbs[p][:], ident_sb[:])
                psTs.append(psT)
            for p in range(4):
                kvT = small.tile([128, 128], F16, name="kvT", tag="kvT", bufs=4)
                nc.scalar.copy(kvT[:], psTs[p][:])
                kvTs.append(kvT)
            for p in range(4):
                for half in range(2):
                    hs = slice(half * 512, (half + 1) * 512)
                    pw = pbig.tile([128, 512], F32, name="ps_w", tag="big")
                    nc.tensor.matmul(
                        pw[:], kvTs[p][:], wp_sb[:, p, hs], start=True, stop=True
                    )
                    if half == 0:
                        nc.vector.tensor_copy(out=wkvp_sb[:, p, hs], in_=pw[:])
                    else:
                        nc.scalar.copy(wkvp_sb[:, p, hs], pw[:])

            pkv.release()
            pnrm = tc.alloc_tile_pool(name="pnrm", bufs=2, space="PSUM")

            # ================= pass B: q' -> normalize -> proj =================
            def emit_pj(blk, q2s, oc_range, split=False):
                ns = slice(blk * BLK, (blk + 1) * BLK)
                for oc in oc_range:
                    pj = pbig.tile([128, BLK], F32, name="ps_pj", tag="big")
                    for p in range(4):
                        nc.tensor.matmul(
                            pj[:],
                            wkvp_sb[:, p, oc * 128 : (oc + 1) * 128],
                            q2s[p][:],
                            start=(p == 0),
                            stop=(p == 3),
                        )
                    pj_sb = small.tile([128, BLK], F16, name="pj_sb", tag="pj", bufs=6)
                    eng = (nc.sync, nc.scalar, nc.gpsimd)[oc % 3]
                    if split:
                        # halve the copy->DMA chain so the last transfers
                        # start as early as possible
                        h = BLK // 2
                        nc.vector.tensor_copy(out=pj_sb[:, 0:h], in_=pj[:, 0:h])
                        eng.dma_start(
                            out_v[:, oc, blk * BLK : blk * BLK + h], pj_sb[:, 0:h]
                        )
                        nc.scalar.copy(pj_sb[:, h:BLK], pj[:, h:BLK])
                        eng.dma_start(
                            out_v[:, oc, blk * BLK + h : (blk + 1) * BLK],
                            pj_sb[:, h:BLK],
                        )
                    else:
                        if oc % 2 == 0:
                            nc.vector.tensor_copy(out=pj_sb[:], in_=pj[:])
                        else:
                            nc.scalar.copy(pj_sb[:], pj[:])
                        eng.dma_start(out_v[:, oc, ns], pj_sb[:])

            def emit_nrm(blk):
                # merged normalizer+broadcast: one matmul per pair gives the
                # per-head norm already broadcast over its 64 partitions
                qp_sb = qps[blk]
                q2s = []
                for p in range(4):
                    nrm = pnrm.tile([128, BLK], F32, name="nrm", tag="nrm")
                    nc.tensor.matmul(
                        nrm[:], ksbc[:, p, :], qp_sb[:, p, :], start=True, stop=True
                    )
                    rec = small.tile([128, BLK], F32, name="rec", tag="rec", bufs=3)
                    nc.vector.reciprocal_approx_fast(out=rec[:], in_=nrm[:])
                    q2 = small.tile([128, BLK], F16, name="q2", tag="q2", bufs=9)
                    nc.vector.tensor_mul(q2[:], qp_sb[:, p, :], rec[:])
                    q2s.append(q2)
                return q2s

            prev_q2 = None  # (blk, q2s) whose proj is still pending
            for blk in range(NBLK - 1):
                q2s = emit_nrm(blk)
                if blk + 4 < NBLK:
                    qps.append(emit_qp(blk + 4))
                if prev_q2 is not None:
                    emit_pj(prev_q2[0], prev_q2[1], range(8))
                prev_q2 = (blk, q2s)

            # last block: interleave the two pending projs so the final
            # out-DMAs start ~4us earlier and the DMA rings drain in time
            q2s7 = emit_nrm(NBLK - 1)
            emit_pj(prev_q2[0], prev_q2[1], range(0, 4))
            emit_pj(NBLK - 1, q2s7, range(0, 2))
            emit_pj(prev_q2[0], prev_q2[1], range(4, 8))
            emit_pj(NBLK - 1, q2s7, range(2, 6))
            emit_pj(NBLK - 1, q2s7, range(6, 8), split=True)

            pnrm.release()

    nc.compile()
    return nc


_NC = None


def _get_nc():
    global _NC
    if _NC is None:
        _NC = _build_nc()
    return _NC


def _host_inputs(x, W_qkv, b_qkv, W_proj, b_proj, proj_mat):
    x = np.asarray(x, dtype=np.float32)
    W_qkv = np.asarray(W_qkv, dtype=np.float32)
    b_qkv = np.asarray(b_qkv, dtype=np.float32)
    W_proj = np.asarray(W_proj, dtype=np.float32)
    proj_mat = np.asarray(proj_mat, dtype=np.float32)

    pt = (proj_mat.T * SCALE).astype(np.float32)  # [hd, F]

    def tile_x(xb):
        # [NBLK, 128, 8, BLK]: contiguous per-partition runs for fast DMA
        xt = xb.T.reshape(8, 128, NBLK, BLK).transpose(2, 1, 0, 3)
        return np.ascontiguousarray(xt)

    def tile_w(w):
        # [D, 512] -> [128, 8, 512]
        return np.ascontiguousarray(w.reshape(8, 128, 512).transpose(1, 0, 2))

    xt8s = [tile_x(x[b]).astype(ml_dtypes.float8_e4m3) for b in range(4)]
    ident = np.eye(128, dtype=np.float16)

    def fuse(Wslc, bslc):
        # W_fused[:, (h f)] = sum_d W.T[:, (h d)] pt[d, f]; bias likewise
        wT = Wslc.T.reshape(D, 8, HD)
        wf = np.einsum("ahd,df->ahf", wT, pt).reshape(D, 512)
        bf = np.einsum("hd,df->hf", bslc.reshape(8, HD), pt).reshape(512)
        return wf, bf

    in_maps = []
    for c in range(8):
        b, g = c // 2, c % 2
        wqs = W_qkv[g * 512 : (g + 1) * 512]
        wks = W_qkv[D + g * 512 : D + (g + 1) * 512]
        wvs = W_qkv[2 * D + g * 512 : 2 * D + (g + 1) * 512]
        bqs = b_qkv[g * 512 : (g + 1) * 512]
        bks = b_qkv[D + g * 512 : D + (g + 1) * 512]
        bvs = b_qkv[2 * D + g * 512 : 2 * D + (g + 1) * 512]
        wqp, bqp = fuse(wqs, bqs)
        wkp, bkp = fuse(wks, bks)
        wp_loc = W_proj[:, g * 512 : (g + 1) * 512].T  # [512 vd, 1024 od]
        in_maps.append(
            {
                "xt8": xt8s[b],
                "wqp8": tile_w(wqp * WS).astype(ml_dtypes.float8_e4m3),
                "wkp8": tile_w(wkp * WS).astype(ml_dtypes.float8_e4m3),
                "wv8": tile_w(np.ascontiguousarray(wvs.T) * WS).astype(
                    ml_dtypes.float8_e4m3
                ),
                "wp": np.ascontiguousarray(
                    wp_loc.reshape(4, 128, 1024).transpose(1, 0, 2)
                ).astype(np.float16),
                "bqpe": np.ascontiguousarray(
                    (bqp + EPS).reshape(4, 128).T
                ).astype(np.float32),
                "bkpb": np.ascontiguousarray(
                    np.broadcast_to(bkp.reshape(1, 8, 64) * WS, (128, 8, 64))
                ).astype(np.float16),
                "ident": ident,
            }
        )
    return in_maps


def kernel(x, W_qkv, b_qkv, W_proj, b_proj, proj_mat):
    b_proj = np.asarray(b_proj, dtype=np.float32)
    b_qkv = np.asarray(b_qkv, dtype=np.float32)
    W_proj = np.asarray(W_proj, dtype=np.float32)
    in_maps = _host_inputs(x, W_qkv, b_qkv, W_proj, b_proj, proj_mat)
    nc = _get_nc()
    res = run_bass_kernel_spmd(nc, in_maps, core_ids=list(range(8)))
    # softmax weights sum to 1, so the v-bias passes through attention
    # unchanged and lands as the constant bv @ Wp^T
    b_eff = b_proj + b_qkv[2 * D :] @ W_proj.T
    final = np.empty((4, N, D), dtype=np.float32)
    for b in range(4):
        acc = res.results[2 * b]["out"].astype(np.float32) + res.results[
            2 * b + 1
        ]["out"].astype(np.float32)
        final[b] = acc.T + b_eff[None, :]
    return final
